# revision 10
# baseline (speedup 1.0000x reference)
"""Trainium2 kernel for nn_AttentionRotationBlock.

Fully on-device 8-core SPMD implementation (Bass/Tile):
  - Phase A (token-parallel): per-core rmsnorm1 stats on its 512-token
    slice; rstd scalars exchanged via a tiny AllGather (2 KiB/core).
    The affine rmsnorm folds into the qkv GEMM:
      qkv = rstd * (x @ (W*gamma)^T) + W@beta.
  - Phase B (head-parallel): each core computes q,k,v for its 2 heads
    x 2 batches over all tokens (exactly 1/8 of the qkv GEMM), causal
    attention with no-max-subtraction exp (scores are provably small),
    softmax denominators via a ones-column appended to V, then ships
    its attention output (1 MiB bf16) through an AllToAll.
  - Phase C (token-parallel): o-projection from the gathered heads,
    residual, rmsnorm2, 3 dense Givens-rotation GEMMs + silu, output.
All large GEMMs run in bf16 with fp32 PSUM accumulation (validated
rel-l2 ~5e-3 vs the fp32 reference). Falls back to a pure-numpy path
if the device path fails.
"""

import sys

import numpy as np

B, T, D, H, NPASS = 2, 2048, 1024, 16, 3
HD = D // H
NC = 8
TOK = B * T            # 4096 tokens
TPC = TOK // NC        # 512 tokens per core
EPS = float(np.finfo(np.float32).eps)


# ---------------------------------------------------------------- host math
def _rmsnorm(x, w):
    ms = np.mean(x * x, axis=-1, keepdims=True)
    return x * (1.0 / np.sqrt(ms + EPS)) * w


def _giv_mats(angles, pi, pj, gate):
    """Dense [D,D] matrices G st rotated = r @ G, with gate folded in."""
    mats = []
    for p in range(NPASS):
        G = np.eye(D, dtype=np.float64)
        ca = np.cos(angles[p].astype(np.float64))
        sa = np.sin(angles[p].astype(np.float64))
        ii = pi[p].astype(np.int64)
        jj = pj[p].astype(np.int64)
        G[ii, ii] = ca
        G[jj, ii] = -sa
        G[ii, jj] = sa
        G[jj, jj] = ca
        G = G * gate[p].astype(np.float64)[None, :]
        mats.append(G.astype(np.float32))
    return mats


def _host_fallback(x, scale_gamma, scale_beta, qkv_w, o_w, norm1_w, norm2_w,
                   angles, gate, bias, pi, pj):
    h = _rmsnorm(x, norm1_w) * scale_gamma + scale_beta
    qkv = (h.reshape(TOK, D) @ qkv_w.T).reshape(B, T, 3, H, HD)
    q = np.moveaxis(qkv[:, :, 0], 1, 2)
    k = np.moveaxis(qkv[:, :, 1], 1, 2)
    v = np.moveaxis(qkv[:, :, 2], 1, 2)
    scale = 1.0 / np.sqrt(HD)
    causal = np.tril(np.ones((T, T), bool))
    out = np.empty((B, H, T, HD), np.float32)
    for b in range(B):
        for hh in range(H):
            s = (q[b, hh] @ k[b, hh].T) * scale
            s = np.where(causal, s, -np.inf).astype(np.float32)
            s -= s.max(axis=-1, keepdims=True)
            e = np.exp(s)
            out[b, hh] = (e / e.sum(axis=-1, keepdims=True)) @ v[b, hh]
    ao = np.swapaxes(out, 1, 2).reshape(B, T, D).astype(np.float32)
    x2 = x + (ao.reshape(TOK, D) @ o_w.T).reshape(B, T, D)
    h2 = _rmsnorm(x2, norm2_w) * scale_gamma + scale_beta
    r = h2.reshape(TOK, D)
    for p, G in enumerate(_giv_mats(angles, pi, pj, gate)):
        r = r @ G + bias[p][None, :]
        r = r * (1.0 / (1.0 + np.exp(-r)))
    return (x2 + r.reshape(B, T, D) - h2).astype(np.float32)


# ---------------------------------------------------------------- device
def _build():
    sys.path.insert(0, "/opt/trn_rl_repo")
    import concourse.bacc as bacc
    import concourse.mybir as mybir
    import concourse.tile as tile
    from concourse.masks import make_identity, make_upper_triangular

    f32 = mybir.dt.float32
    bf16 = mybir.dt.bfloat16
    AF = mybir.ActivationFunctionType
    OP = mybir.AluOpType

    nc = bacc.Bacc(None, num_devices=NC)

    xt = nc.dram_tensor("xt", [D, TOK], bf16, kind="ExternalInput")
    xs = nc.dram_tensor("xs", [TPC, D], f32, kind="ExternalInput")
    wgt = nc.dram_tensor("wgt", [D, 384], bf16, kind="ExternalInput")
    bwr = nc.dram_tensor("bwr", [128, 3], f32, kind="ExternalInput")
    owt = nc.dram_tensor("owt", [D, D], bf16, kind="ExternalInput")
    gm = nc.dram_tensor("gm", [NPASS, D, D], bf16, kind="ExternalInput")
    b2r = nc.dram_tensor("b2r", [128, NPASS, 8], f32, kind="ExternalInput")
    gamr = nc.dram_tensor("gamr", [128, 8], f32, kind="ExternalInput")
    betr = nc.dram_tensor("betr", [128, 8], f32, kind="ExternalInput")
    yt = nc.dram_tensor("yt", [D, TPC], f32, kind="ExternalOutput")

    with tile.TileContext(nc) as tc:
        with (
            tc.tile_pool(name="consts", bufs=1) as consts,
            tc.tile_pool(name="acts", bufs=1) as acts,
            tc.tile_pool(name="xch", bufs=2) as xchp,
            tc.tile_pool(name="gmp", bufs=2) as gmp,
            tc.tile_pool(name="sqp", bufs=1) as sqp,
            tc.tile_pool(name="tmp", bufs=3) as tmp,
            tc.tile_pool(name="etmp", bufs=3) as etmp,
            tc.tile_pool(name="rbp", bufs=2) as rbp,
            tc.tile_pool(name="att", bufs=2) as att,
            tc.tile_pool(name="stats", bufs=1) as stats,
            tc.tile_pool(name="ps_mm", bufs=3, space="PSUM") as ps_mm,
            tc.tile_pool(name="ps_s", bufs=3, space="PSUM") as ps_s,
            tc.tile_pool(name="ps_o", bufs=2, space="PSUM") as ps_o,
            tc.tile_pool(name="dram", bufs=1, space="DRAM") as dram,
        ):
            # ---------------- consts
            epsb = consts.tile([128, 1], f32, tag="epsb")
            nc.vector.memset(epsb[:, :], EPS)
            identf = consts.tile([128, 128], f32, tag="identf")
            make_identity(nc, identf[:, :])
            identb = consts.tile([128, 128], bf16, tag="identb")
            make_identity(nc, identb[:, :])
            trimask = consts.tile([128, 128], bf16, tag="trimask")
            make_upper_triangular(nc, trimask[:, :], val=1.0, diag=True)

            # ---------------- phase A: own-slice rstd1 + AllGather
            x_nat = acts.tile([128, 4, D], f32, tag="bigA")
            for tt in range(4):
                nc.sync.dma_start(out=x_nat[:, tt, :],
                                  in_=xs[tt * 128:(tt + 1) * 128, :])
            ssq = stats.tile([128, 4], f32, tag="ssq")
            for tt in range(4):
                sq = sqp.tile([128, D], f32, tag="sq")
                nc.scalar.activation(out=sq[:, :], in_=x_nat[:, tt, :],
                                     func=AF.Square,
                                     accum_out=ssq[:, tt:tt + 1])
            std = stats.tile([128, 4], f32, tag="std")
            nc.scalar.activation(out=std[:, :], in_=ssq[:, :], func=AF.Sqrt,
                                 scale=1.0 / D, bias=epsb[:, 0:1])
            rstd1 = stats.tile([128, 4], f32, tag="rstd1")
            nc.vector.reciprocal(out=rstd1[:, :], in_=std[:, :])

            rs_in = dram.tile([TPC, 1], f32)
            rs_out = dram.tile([NC, TPC], f32)
            for tt in range(4):
                nc.sync.dma_start(out=rs_in[tt * 128:(tt + 1) * 128, 0:1],
                                  in_=rstd1[:, tt:tt + 1])
            nc.gpsimd.collective_compute(
                "AllGather", OP.bypass, replica_groups=[list(range(NC))],
                ins=[rs_in.opt()], outs=[rs_out.opt()])

            rstdK = consts.tile([128, 32], f32, tag="rstdK")
            nc.sync.dma_start(
                out=rstdK[:, :],
                in_=rs_out[:, :].rearrange("r (kl p) -> p (r kl)", p=128))

            # ---------------- phase B1: qkv slice GEMM (2 heads, all tokens)
            wgt_sb = acts.tile([128, 8, 384], bf16, tag="wgt")
            nc.sync.dma_start(
                out=wgt_sb[:, :, :],
                in_=wgt[:, :].rearrange("(k p) j -> p k j", p=128))
            bw_sb = consts.tile([128, 3], f32, tag="bw")
            nc.sync.dma_start(out=bw_sb[:, :], in_=bwr[:, :])

            qT = acts.tile([128, TOK], bf16, tag="bigC")
            kT = acts.tile([128, TOK], bf16, tag="bigD")
            vT = acts.tile([128, TOK], bf16, tag="bigE")
            for tb in range(8):
                xck = xchp.tile([128, 8, 512], bf16, tag="xck")
                nc.sync.dma_start(
                    out=xck[:, :, :],
                    in_=xt[:, tb * 512:(tb + 1) * 512]
                    .rearrange("(k p) t -> p k t", p=128))
                rrow = stats.tile([1, 512], f32, tag="rrow")
                nc.sync.dma_start(out=rrow[:, :], in_=rs_out[tb:tb + 1, :])
                rsb = rbp.tile([128, 512], f32, tag="rsb")
                nc.gpsimd.partition_broadcast(rsb[:, :], rrow[:1, :])
                sl = slice(tb * 512, (tb + 1) * 512)
                for j, dest in enumerate((qT, kT, vT)):
                    pq = ps_mm.tile([128, 512], f32, tag="mm")
                    for dk in range(8):
                        nc.tensor.matmul(
                            pq[:, :], wgt_sb[:, dk, j * 128:(j + 1) * 128],
                            xck[:, dk, :], start=(dk == 0), stop=(dk == 7))
                    if j == 1:  # k: bias only (rstd_k folded into exp scale)
                        nc.scalar.activation(out=dest[:, sl], in_=pq[:, :],
                                             func=AF.Identity,
                                             bias=bw_sb[:, 1:2])
                    else:
                        tq = tmp.tile([128, 512], f32, tag="t5")
                        nc.scalar.activation(out=tq[:, :], in_=pq[:, :],
                                             func=AF.Identity,
                                             bias=bw_sb[:, j:j + 1])
                        nc.vector.tensor_tensor(out=dest[:, sl], in0=tq[:, :],
                                                in1=rsb[:, :], op=OP.mult)

            # ---------------- phase B2: v transpose -> [tok, hd]+ones
            v_stat = acts.tile([128, 64, 65], bf16, tag="v_stat")
            nc.vector.memset(v_stat[:, :, :], 1.0)
            for b in range(2):
                for kt in range(16):
                    pt = ps_mm.tile([128, 256], bf16, tag="mm")
                    nc.tensor.transpose(
                        pt[:, :128],
                        vT[:, b * T + kt * 128:b * T + kt * 128 + 128],
                        identb[:, :])
                    for hh in range(2):
                        idx = (b * 2 + hh) * 16 + kt
                        nc.scalar.activation(
                            out=v_stat[:, idx, 0:64],
                            in_=pt[:, hh * 64:(hh + 1) * 64], func=AF.Copy)

            # ---------------- phase B3: causal attention
            a2a_in = dram.tile([NC, 128, 512], bf16)
            a2a_out = dram.tile([NC, 128, 512], bf16)
            for bh in range(4):
                b, hh = bh >> 1, bh & 1
                rows = slice(hh * 64, (hh + 1) * 64)
                for qc in range(4):
                    q0 = b * T + qc * 512
                    dst = b * 4 + qc
                    o_ps = ps_o.tile([65, 512], f32, tag="ops")
                    n_kt = 4 * (qc + 1)
                    for kt in range(n_kt):
                        band_j = kt - 4 * qc
                        col0 = max(0, band_j * 128)
                        n = 512 - col0
                        s_ps = ps_s.tile([128, 512], f32, tag="sps")
                        nc.tensor.matmul(
                            s_ps[:, :n],
                            kT[rows, b * T + kt * 128:b * T + kt * 128 + 128],
                            qT[rows, q0 + col0:q0 + 512],
                            start=True, stop=True)
                        e_sb = etmp.tile([128, 512], bf16, tag="esb")
                        gkt = b * 16 + kt
                        nc.scalar.activation(out=e_sb[:, :n], in_=s_ps[:, :n],
                                             func=AF.Exp,
                                             scale=rstdK[:, gkt:gkt + 1])
                        if band_j >= 0:
                            nc.vector.tensor_tensor(
                                out=e_sb[:, 0:128], in0=e_sb[:, 0:128],
                                in1=trimask[:, :], op=OP.mult)
                        nc.tensor.matmul(
                            o_ps[:, col0:512], v_stat[:, bh * 16 + kt, :],
                            e_sb[:, :n], start=(kt == 0),
                            stop=(kt == n_kt - 1), skip_group_check=True)
                    srow = att.tile([1, 512], f32, tag="srow")
                    nc.scalar.activation(out=srow[:, :], in_=o_ps[64:65, :],
                                         func=AF.Copy)
                    rrow2 = att.tile([1, 512], f32, tag="rrow2")
                    nc.vector.reciprocal(out=rrow2[:, :], in_=srow[:, :])
                    rbc = att.tile([64, 512], f32, tag="rbc")
                    nc.gpsimd.partition_broadcast(rbc[:, :], rrow2[:1, :])
                    ao = att.tile([64, 512], bf16, tag="ao")
                    nc.vector.tensor_tensor(out=ao[:, :], in0=o_ps[0:64, :],
                                            in1=rbc[:, :], op=OP.mult)
                    nc.sync.dma_start(
                        out=a2a_in[dst, hh * 64:(hh + 1) * 64, :],
                        in_=ao[:, :])

            # ---------------- phase B4: AllToAll of attention outputs
            nc.gpsimd.collective_compute(
                "AllToAll", OP.bypass, replica_groups=[list(range(NC))],
                ins=[a2a_in.opt()], outs=[a2a_out.opt()])

            # ---------------- phase C1: o-proj + residual (natural layout)
            aosb = acts.tile([128, 8, 512], bf16, tag="bigC")
            for r in range(NC):
                nc.sync.dma_start(out=aosb[:, r, :], in_=a2a_out[r, :, :])
            owt_lo = xchp.tile([128, 8, 512], bf16, tag="xck")
            owt_hi = xchp.tile([128, 8, 512], bf16, tag="xck")
            for oc, ow_sb in enumerate((owt_lo, owt_hi)):
                nc.sync.dma_start(
                    out=ow_sb[:, :, :],
                    in_=owt[:, oc * 512:(oc + 1) * 512]
                    .rearrange("(k p) j -> p k j", p=128))
            for tt in range(4):
                for oc, ow_sb in enumerate((owt_lo, owt_hi)):
                    po = ps_mm.tile([128, 512], f32, tag="mm")
                    for r in range(NC):
                        nc.tensor.matmul(
                            po[:, :], aosb[:, r, tt * 128:(tt + 1) * 128],
                            ow_sb[:, r, :],
                            start=(r == 0), stop=(r == NC - 1))
                    osl = slice(oc * 512, (oc + 1) * 512)
                    nc.vector.tensor_tensor(out=x_nat[:, tt, osl],
                                            in0=po[:, :],
                                            in1=x_nat[:, tt, osl], op=OP.add)

            # ---------------- phase C2: rstd2 + transpose to [D, tok]
            ssq2 = stats.tile([128, 4], f32, tag="ssq2")
            for tt in range(4):
                sq2 = sqp.tile([128, D], f32, tag="sq")
                nc.scalar.activation(out=sq2[:, :], in_=x_nat[:, tt, :],
                                     func=AF.Square,
                                     accum_out=ssq2[:, tt:tt + 1])
            std2 = stats.tile([128, 4], f32, tag="std2")
            nc.scalar.activation(out=std2[:, :], in_=ssq2[:, :], func=AF.Sqrt,
                                 scale=1.0 / D, bias=epsb[:, 0:1])
            rstd2 = stats.tile([128, 4], f32, tag="rstd2")
            nc.vector.reciprocal(out=rstd2[:, :], in_=std2[:, :])
            rs2d = dram.tile([TPC, 1], f32)
            for tt in range(4):
                nc.sync.dma_start(out=rs2d[tt * 128:(tt + 1) * 128, 0:1],
                                  in_=rstd2[:, tt:tt + 1])
            r2row = stats.tile([1, 512], f32, tag="r2row")
            nc.sync.dma_start(out=r2row[:, :],
                              in_=rs2d[:, :].rearrange("t one -> (t one)"))
            rstd2B = consts.tile([128, 512], f32, tag="rstd2B")
            nc.gpsimd.partition_broadcast(rstd2B[:, :], r2row[:1, :])

            x2T = acts.tile([128, 8, 512], f32, tag="x2T")
            for tt in range(4):
                for dk in range(8):
                    ptr = ps_mm.tile([128, 512], f32, tag="mm")
                    nc.tensor.transpose(
                        ptr[:, :128], x_nat[:, tt, dk * 128:(dk + 1) * 128],
                        identf[:, :])
                    nc.vector.tensor_copy(
                        out=x2T[:, dk, tt * 128:(tt + 1) * 128],
                        in_=ptr[:, :128])

            gam_sb = consts.tile([128, 8], f32, tag="gam")
            nc.sync.dma_start(out=gam_sb[:, :], in_=gamr[:, :])
            bet_sb = consts.tile([128, 8], f32, tag="bet")
            nc.sync.dma_start(out=bet_sb[:, :], in_=betr[:, :])
            b2_sb = consts.tile([128, NPASS, 8], f32, tag="b2")
            nc.sync.dma_start(out=b2_sb[:, :, :], in_=b2r[:, :, :])

            h2T = acts.tile([128, 8, 512], bf16, tag="bigE")
            for dk in range(8):
                th = tmp.tile([128, 512], f32, tag="t5")
                nc.vector.tensor_tensor(out=th[:, :], in0=x2T[:, dk, :],
                                        in1=rstd2B[:, :], op=OP.mult)
                nc.vector.tensor_scalar(
                    out=h2T[:, dk, :], in0=th[:, :],
                    scalar1=gam_sb[:, dk:dk + 1], scalar2=bet_sb[:, dk:dk + 1],
                    op0=OP.mult, op1=OP.add)

            # ---------------- phase C3: rotation passes
            rAB = acts.tile([128, 2, 8, 512], bf16, tag="bigA")
            cur = h2T
            for p in range(NPASS):
                gsb = gmp.tile([128, 8, D], bf16, tag="g")
                nc.sync.dma_start(
                    out=gsb[:, :, :],
                    in_=gm[p, :, :].rearrange("(ki pp) j -> pp ki j", pp=128))
                for jc in range(8):
                    pr = ps_mm.tile([128, 512], f32, tag="mm")
                    for ki in range(8):
                        nc.tensor.matmul(
                            pr[:, :], gsb[:, ki, jc * 128:(jc + 1) * 128],
                            cur[:, ki, :] if p == 0
                            else cur[:, p & 1 ^ 1, ki, :],
                            start=(ki == 0), stop=(ki == 7))
                    dst_half = p & 1
                    nc.scalar.activation(out=rAB[:, dst_half, jc, :],
                                         in_=pr[:, :], func=AF.Silu,
                                         bias=b2_sb[:, p, jc:jc + 1])
                cur = rAB
            # after 3 passes result is in rAB[:, 0] (p=2 wrote half 0)

            # ---------------- phase C4: y = x2 + r - h2  (T layout out)
            for dk in range(8):
                ty = tmp.tile([128, 512], f32, tag="t5")
                nc.vector.tensor_tensor(out=ty[:, :], in0=rAB[:, 0, dk, :],
                                        in1=h2T[:, dk, :], op=OP.subtract)
                nc.vector.tensor_tensor(out=ty[:, :], in0=ty[:, :],
                                        in1=x2T[:, dk, :], op=OP.add)
                nc.sync.dma_start(out=yt[dk * 128:(dk + 1) * 128, :],
                                  in_=ty[:, :])
    nc.finalize()
    return nc


_NC_CACHE = [None]


def _device_run(x, gamma1, gamma2, scale_beta, qkv_w, o_w, angles, gate,
                bias, pi, pj):
    sys.path.insert(0, "/opt/trn_rl_repo")
    import ml_dtypes
    from concourse import bass_utils

    bf = ml_dtypes.bfloat16
    if _NC_CACHE[0] is None:
        _NC_CACHE[0] = _build()
    nc = _NC_CACHE[0]

    xf = x.reshape(TOK, D).astype(np.float32)
    xt = np.ascontiguousarray(xf.T).astype(bf)

    Wg = qkv_w * gamma1[None, :]
    bW = qkv_w @ scale_beta
    gmats = _giv_mats(angles, pi, pj, gate)
    gm = np.stack(gmats).astype(bf)
    owt = np.ascontiguousarray(o_w.T).astype(bf)
    b2r = np.ascontiguousarray(
        bias.astype(np.float32).reshape(NPASS, 8, 128).transpose(2, 0, 1))
    gamr = np.ascontiguousarray(
        gamma2.astype(np.float32).reshape(8, 128).T)
    betr = np.ascontiguousarray(
        scale_beta.astype(np.float32).reshape(8, 128).T)

    shared = {"xt": xt, "owt": owt, "gm": gm, "b2r": b2r, "gamr": gamr,
              "betr": betr}
    in_maps = []
    for c in range(NC):
        rq = slice(128 * c, 128 * (c + 1))
        rk = slice(D + 128 * c, D + 128 * (c + 1))
        rv = slice(2 * D + 128 * c, 2 * D + 128 * (c + 1))
        wg_slice = np.concatenate(
            [Wg[rq], Wg[rk] / np.sqrt(HD), Wg[rv]], axis=0)
        bw_slice = np.concatenate(
            [bW[rq], bW[rk] / np.sqrt(HD), bW[rv]], axis=0)
        m = dict(shared)
        m["wgt"] = np.ascontiguousarray(wg_slice.T).astype(bf)
        m["bwr"] = np.ascontiguousarray(
            bw_slice.astype(np.float32).reshape(3, 128).T)
        m["xs"] = np.ascontiguousarray(xf[c * TPC:(c + 1) * TPC])
        in_maps.append(m)

    res = bass_utils.run_bass_kernel_spmd(nc, in_maps,
                                          core_ids=list(range(NC)))
    yf = np.empty((TOK, D), np.float32)
    for c in range(NC):
        yf[c * TPC:(c + 1) * TPC] = res.results[c]["yt"].T
    return yf.reshape(B, T, D)


def kernel(x, scale_gamma, scale_beta, qkv_w, o_w, norm1_w, norm2_w,
           angles, gate, bias, pi, pj):
    x = np.asarray(x, np.float32)
    args = (np.asarray(scale_gamma, np.float32),
            np.asarray(scale_beta, np.float32),
            np.asarray(qkv_w, np.float32), np.asarray(o_w, np.float32))
    rot = (np.asarray(angles, np.float32), np.asarray(gate, np.float32),
           np.asarray(bias, np.float32), np.asarray(pi), np.asarray(pj))
    g1 = args[0] * np.asarray(norm1_w, np.float32)
    g2 = args[0] * np.asarray(norm2_w, np.float32)
    try:
        return _device_run(x, g1, g2, args[1], args[2], args[3],
                           rot[0], rot[1], rot[2], rot[3], rot[4])
    except Exception as e:  # pragma: no cover - safety net
        print(f"device path failed ({type(e).__name__}: {e}); "
              "using host fallback", file=sys.stderr)
        return _host_fallback(x, args[0], args[1], args[2], args[3],
                              np.asarray(norm1_w, np.float32),
                              np.asarray(norm2_w, np.float32),
                              rot[0], rot[1], rot[2], rot[3], rot[4])


# revision 11
# speedup vs baseline: 1.0913x; 1.0913x over previous
"""Trainium2 kernel for nn_AttentionRotationBlock.

Fully on-device 8-core SPMD implementation (Bass/Tile):
  - Phase A (token-parallel): per-core rmsnorm1 stats on its 512-token
    slice; rstd scalars exchanged via a tiny AllGather (2 KiB/core).
    The affine rmsnorm folds into the qkv GEMM:
      qkv = rstd * (x @ (W*gamma)^T) + W@beta.
  - Phase B (head-parallel): each core computes q,k,v for its 2 heads
    x 2 batches over all tokens (exactly 1/8 of the qkv GEMM), causal
    attention with no-max-subtraction exp (scores are provably small),
    softmax denominators via a ones-column appended to V, then ships
    its attention output (1 MiB bf16) through an AllToAll.
  - Phase C (token-parallel): o-projection from the gathered heads,
    residual, rmsnorm2, 3 dense Givens-rotation GEMMs + silu, output.
All large GEMMs run in bf16 with fp32 PSUM accumulation (validated
rel-l2 ~5e-3 vs the fp32 reference). Falls back to a pure-numpy path
if the device path fails.
"""

import sys

import numpy as np

B, T, D, H, NPASS = 2, 2048, 1024, 16, 3
HD = D // H
NC = 8
TOK = B * T            # 4096 tokens
TPC = TOK // NC        # 512 tokens per core
EPS = float(np.finfo(np.float32).eps)


# ---------------------------------------------------------------- host math
def _rmsnorm(x, w):
    ms = np.mean(x * x, axis=-1, keepdims=True)
    return x * (1.0 / np.sqrt(ms + EPS)) * w


def _giv_mats(angles, pi, pj, gate):
    """Dense [D,D] matrices G st rotated = r @ G, with gate folded in."""
    mats = []
    for p in range(NPASS):
        G = np.eye(D, dtype=np.float64)
        ca = np.cos(angles[p].astype(np.float64))
        sa = np.sin(angles[p].astype(np.float64))
        ii = pi[p].astype(np.int64)
        jj = pj[p].astype(np.int64)
        G[ii, ii] = ca
        G[jj, ii] = -sa
        G[ii, jj] = sa
        G[jj, jj] = ca
        G = G * gate[p].astype(np.float64)[None, :]
        mats.append(G.astype(np.float32))
    return mats


def _host_fallback(x, scale_gamma, scale_beta, qkv_w, o_w, norm1_w, norm2_w,
                   angles, gate, bias, pi, pj):
    h = _rmsnorm(x, norm1_w) * scale_gamma + scale_beta
    qkv = (h.reshape(TOK, D) @ qkv_w.T).reshape(B, T, 3, H, HD)
    q = np.moveaxis(qkv[:, :, 0], 1, 2)
    k = np.moveaxis(qkv[:, :, 1], 1, 2)
    v = np.moveaxis(qkv[:, :, 2], 1, 2)
    scale = 1.0 / np.sqrt(HD)
    causal = np.tril(np.ones((T, T), bool))
    out = np.empty((B, H, T, HD), np.float32)
    for b in range(B):
        for hh in range(H):
            s = (q[b, hh] @ k[b, hh].T) * scale
            s = np.where(causal, s, -np.inf).astype(np.float32)
            s -= s.max(axis=-1, keepdims=True)
            e = np.exp(s)
            out[b, hh] = (e / e.sum(axis=-1, keepdims=True)) @ v[b, hh]
    ao = np.swapaxes(out, 1, 2).reshape(B, T, D).astype(np.float32)
    x2 = x + (ao.reshape(TOK, D) @ o_w.T).reshape(B, T, D)
    h2 = _rmsnorm(x2, norm2_w) * scale_gamma + scale_beta
    r = h2.reshape(TOK, D)
    for p, G in enumerate(_giv_mats(angles, pi, pj, gate)):
        r = r @ G + bias[p][None, :]
        r = r * (1.0 / (1.0 + np.exp(-r)))
    return (x2 + r.reshape(B, T, D) - h2).astype(np.float32)


# ---------------------------------------------------------------- device
def _build():
    sys.path.insert(0, "/opt/trn_rl_repo")
    import concourse.bacc as bacc
    import concourse.mybir as mybir
    import concourse.tile as tile
    from concourse.masks import make_identity, make_upper_triangular

    f32 = mybir.dt.float32
    bf16 = mybir.dt.bfloat16
    AF = mybir.ActivationFunctionType
    OP = mybir.AluOpType

    nc = bacc.Bacc(None, num_devices=NC)

    xt = nc.dram_tensor("xt", [D, TOK], bf16, kind="ExternalInput")
    xs = nc.dram_tensor("xs", [TPC, D], f32, kind="ExternalInput")
    wgt = nc.dram_tensor("wgt", [D, 384], bf16, kind="ExternalInput")
    bwr = nc.dram_tensor("bwr", [128, 3], f32, kind="ExternalInput")
    owt = nc.dram_tensor("owt", [D, D], bf16, kind="ExternalInput")
    gm = nc.dram_tensor("gm", [NPASS, D, D], bf16, kind="ExternalInput")
    b2r = nc.dram_tensor("b2r", [128, NPASS, 8], f32, kind="ExternalInput")
    gamr = nc.dram_tensor("gamr", [128, 8], f32, kind="ExternalInput")
    betr = nc.dram_tensor("betr", [128, 8], f32, kind="ExternalInput")
    yt = nc.dram_tensor("yt", [D, TPC], f32, kind="ExternalOutput")

    with tile.TileContext(nc) as tc:
        with (
            tc.tile_pool(name="consts", bufs=1) as consts,
            tc.tile_pool(name="acts", bufs=1) as acts,
            tc.tile_pool(name="xch", bufs=2) as xchp,
            tc.tile_pool(name="gmp", bufs=2) as gmp,
            tc.tile_pool(name="sqp", bufs=1) as sqp,
            tc.tile_pool(name="tmp", bufs=3) as tmp,
            tc.tile_pool(name="etmp", bufs=3) as etmp,
            tc.tile_pool(name="rbp", bufs=2) as rbp,
            tc.tile_pool(name="att", bufs=2) as att,
            tc.tile_pool(name="stats", bufs=1) as stats,
            tc.tile_pool(name="ps_mm", bufs=3, space="PSUM") as ps_mm,
            tc.tile_pool(name="ps_s", bufs=3, space="PSUM") as ps_s,
            tc.tile_pool(name="ps_o", bufs=2, space="PSUM") as ps_o,
            tc.tile_pool(name="dram", bufs=1, space="DRAM") as dram,
        ):
            # ---------------- consts
            epsb = consts.tile([128, 1], f32, tag="epsb")
            nc.vector.memset(epsb[:, :], EPS)
            identf = consts.tile([128, 128], f32, tag="identf")
            make_identity(nc, identf[:, :])
            identb = consts.tile([128, 128], bf16, tag="identb")
            make_identity(nc, identb[:, :])
            trimask = consts.tile([128, 128], bf16, tag="trimask")
            make_upper_triangular(nc, trimask[:, :], val=1.0, diag=True)

            # ---------------- phase A: own-slice rstd1 + AllGather
            x_nat = acts.tile([128, 4, D], f32, tag="bigA")
            for tt in range(4):
                nc.sync.dma_start(out=x_nat[:, tt, :],
                                  in_=xs[tt * 128:(tt + 1) * 128, :])
            ssq = stats.tile([128, 4], f32, tag="ssq")
            for tt in range(4):
                sq = sqp.tile([128, D], f32, tag="sq")
                nc.scalar.activation(out=sq[:, :], in_=x_nat[:, tt, :],
                                     func=AF.Square,
                                     accum_out=ssq[:, tt:tt + 1])
            std = stats.tile([128, 4], f32, tag="std")
            nc.scalar.activation(out=std[:, :], in_=ssq[:, :], func=AF.Sqrt,
                                 scale=1.0 / D, bias=epsb[:, 0:1])
            rstd1 = stats.tile([128, 4], f32, tag="rstd1")
            nc.vector.reciprocal(out=rstd1[:, :], in_=std[:, :])

            rs_in = dram.tile([TPC, 1], f32)
            rs_out = dram.tile([NC, TPC], f32)
            for tt in range(4):
                nc.sync.dma_start(out=rs_in[tt * 128:(tt + 1) * 128, 0:1],
                                  in_=rstd1[:, tt:tt + 1])
            nc.gpsimd.collective_compute(
                "AllGather", OP.bypass, replica_groups=[list(range(NC))],
                ins=[rs_in.opt()], outs=[rs_out.opt()])

            rstdK = consts.tile([128, 32], f32, tag="rstdK")
            nc.sync.dma_start(
                out=rstdK[:, :],
                in_=rs_out[:, :].rearrange("r (kl p) -> p (r kl)", p=128))

            # ---------------- phase B1: qkv slice GEMM (2 heads, all tokens)
            wgt_sb = acts.tile([128, 8, 384], bf16, tag="wgt")
            nc.sync.dma_start(
                out=wgt_sb[:, :, :],
                in_=wgt[:, :].rearrange("(k p) j -> p k j", p=128))
            bw_sb = consts.tile([128, 3], f32, tag="bw")
            nc.sync.dma_start(out=bw_sb[:, :], in_=bwr[:, :])

            qT = acts.tile([128, TOK], bf16, tag="bigC")
            kT = acts.tile([128, TOK], bf16, tag="bigD")
            vT = acts.tile([128, TOK], bf16, tag="bigE")
            for tb in range(8):
                xck = xchp.tile([128, 8, 512], bf16, tag="xck")
                nc.sync.dma_start(
                    out=xck[:, :, :],
                    in_=xt[:, tb * 512:(tb + 1) * 512]
                    .rearrange("(k p) t -> p k t", p=128))
                rrow = stats.tile([1, 512], f32, tag="rrow")
                nc.sync.dma_start(out=rrow[:, :], in_=rs_out[tb:tb + 1, :])
                rsb = rbp.tile([128, 512], f32, tag="rsb")
                nc.gpsimd.partition_broadcast(rsb[:, :], rrow[:1, :])
                sl = slice(tb * 512, (tb + 1) * 512)
                for j, dest in enumerate((qT, kT, vT)):
                    pq = ps_mm.tile([128, 512], f32, tag="mm")
                    for dk in range(8):
                        nc.tensor.matmul(
                            pq[:, :], wgt_sb[:, dk, j * 128:(j + 1) * 128],
                            xck[:, dk, :], start=(dk == 0), stop=(dk == 7))
                    if j == 1:  # k: bias only (rstd_k folded into exp scale)
                        nc.scalar.activation(out=dest[:, sl], in_=pq[:, :],
                                             func=AF.Identity,
                                             bias=bw_sb[:, 1:2])
                    else:
                        tq = tmp.tile([128, 512], f32, tag="t5")
                        nc.scalar.activation(out=tq[:, :], in_=pq[:, :],
                                             func=AF.Identity,
                                             bias=bw_sb[:, j:j + 1])
                        nc.vector.tensor_tensor(out=dest[:, sl], in0=tq[:, :],
                                                in1=rsb[:, :], op=OP.mult)

            # ---------------- phase B2: v transpose -> [tok, hd]+ones
            v_stat = acts.tile([128, 64, 65], bf16, tag="v_stat")
            nc.vector.memset(v_stat[:, :, :], 1.0)
            for b in range(2):
                for kt in range(16):
                    pt = ps_mm.tile([128, 256], bf16, tag="mm")
                    nc.tensor.transpose(
                        pt[:, :128],
                        vT[:, b * T + kt * 128:b * T + kt * 128 + 128],
                        identb[:, :])
                    for hh in range(2):
                        idx = (b * 2 + hh) * 16 + kt
                        nc.scalar.activation(
                            out=v_stat[:, idx, 0:64],
                            in_=pt[:, hh * 64:(hh + 1) * 64], func=AF.Copy)

            # ---------------- phase B3: causal attention
            a2a_in = dram.tile([NC, 128, 512], bf16)
            a2a_out = dram.tile([NC, 128, 512], bf16)
            for bh in range(4):
                b, hh = bh >> 1, bh & 1
                rows = slice(hh * 64, (hh + 1) * 64)
                for qc in range(4):
                    q0 = b * T + qc * 512
                    dst = b * 4 + qc
                    o_ps = ps_o.tile([65, 512], f32, tag="ops")
                    n_kt = 4 * (qc + 1)
                    for kt in range(n_kt):
                        band_j = kt - 4 * qc
                        col0 = max(0, band_j * 128)
                        n = 512 - col0
                        s_ps = ps_s.tile([128, 512], f32, tag="sps")
                        nc.tensor.matmul(
                            s_ps[:, :n],
                            kT[rows, b * T + kt * 128:b * T + kt * 128 + 128],
                            qT[rows, q0 + col0:q0 + 512],
                            start=True, stop=True)
                        e_sb = etmp.tile([128, 512], bf16, tag="esb")
                        gkt = b * 16 + kt
                        nc.scalar.activation(out=e_sb[:, :n], in_=s_ps[:, :n],
                                             func=AF.Exp,
                                             scale=rstdK[:, gkt:gkt + 1])
                        if band_j >= 0:
                            nc.vector.tensor_tensor(
                                out=e_sb[:, 0:128], in0=e_sb[:, 0:128],
                                in1=trimask[:, :], op=OP.mult)
                        nc.tensor.matmul(
                            o_ps[:, col0:512], v_stat[:, bh * 16 + kt, :],
                            e_sb[:, :n], start=(kt == 0),
                            stop=(kt == n_kt - 1), skip_group_check=True)
                    srow = att.tile([1, 512], f32, tag="srow")
                    nc.scalar.activation(out=srow[:, :], in_=o_ps[64:65, :],
                                         func=AF.Copy)
                    rrow2 = att.tile([1, 512], f32, tag="rrow2")
                    nc.vector.reciprocal(out=rrow2[:, :], in_=srow[:, :])
                    rbc = att.tile([64, 512], f32, tag="rbc")
                    nc.gpsimd.partition_broadcast(rbc[:, :], rrow2[:1, :])
                    ao = att.tile([64, 512], bf16, tag="ao")
                    nc.vector.tensor_tensor(out=ao[:, :], in0=o_ps[0:64, :],
                                            in1=rbc[:, :], op=OP.mult)
                    nc.sync.dma_start(
                        out=a2a_in[dst, hh * 64:(hh + 1) * 64, :],
                        in_=ao[:, :])

            # ---------------- phase B4: AllToAll of attention outputs
            nc.gpsimd.collective_compute(
                "AllToAll", OP.bypass, replica_groups=[list(range(NC))],
                ins=[a2a_in.opt()], outs=[a2a_out.opt()])

            # ---------------- phase C1: o-proj + residual (natural layout)
            aosb = acts.tile([128, 8, 512], bf16, tag="bigC")
            for r in range(NC):
                nc.sync.dma_start(out=aosb[:, r, :], in_=a2a_out[r, :, :])
            owt_lo = xchp.tile([128, 8, 512], bf16, tag="xck")
            owt_hi = xchp.tile([128, 8, 512], bf16, tag="xck")
            for oc, ow_sb in enumerate((owt_lo, owt_hi)):
                nc.sync.dma_start(
                    out=ow_sb[:, :, :],
                    in_=owt[:, oc * 512:(oc + 1) * 512]
                    .rearrange("(k p) j -> p k j", p=128))
            for tt in range(4):
                for oc, ow_sb in enumerate((owt_lo, owt_hi)):
                    po = ps_mm.tile([128, 512], f32, tag="mm")
                    for r in range(NC):
                        nc.tensor.matmul(
                            po[:, :], aosb[:, r, tt * 128:(tt + 1) * 128],
                            ow_sb[:, r, :],
                            start=(r == 0), stop=(r == NC - 1))
                    osl = slice(oc * 512, (oc + 1) * 512)
                    nc.vector.tensor_tensor(out=x_nat[:, tt, osl],
                                            in0=po[:, :],
                                            in1=x_nat[:, tt, osl], op=OP.add)

            # ---------------- phase C2: rstd2 + transpose to [D, tok]
            ssq2 = stats.tile([128, 4], f32, tag="ssq2")
            for tt in range(4):
                sq2 = sqp.tile([128, D], f32, tag="sq")
                nc.scalar.activation(out=sq2[:, :], in_=x_nat[:, tt, :],
                                     func=AF.Square,
                                     accum_out=ssq2[:, tt:tt + 1])
            std2 = stats.tile([128, 4], f32, tag="std2")
            nc.scalar.activation(out=std2[:, :], in_=ssq2[:, :], func=AF.Sqrt,
                                 scale=1.0 / D, bias=epsb[:, 0:1])
            rstd2 = stats.tile([128, 4], f32, tag="rstd2")
            nc.vector.reciprocal(out=rstd2[:, :], in_=std2[:, :])
            rs2d = dram.tile([TPC, 1], f32)
            for tt in range(4):
                nc.sync.dma_start(out=rs2d[tt * 128:(tt + 1) * 128, 0:1],
                                  in_=rstd2[:, tt:tt + 1])
            r2row = stats.tile([1, 512], f32, tag="r2row")
            nc.sync.dma_start(out=r2row[:, :],
                              in_=rs2d[:, :].rearrange("t one -> (t one)"))
            rstd2B = consts.tile([128, 512], f32, tag="rstd2B")
            nc.gpsimd.partition_broadcast(rstd2B[:, :], r2row[:1, :])

            x2T = acts.tile([128, 8, 512], f32, tag="x2T")
            for tt in range(4):
                for dk in range(8):
                    ptr = ps_mm.tile([128, 512], f32, tag="mm")
                    nc.tensor.transpose(
                        ptr[:, :128], x_nat[:, tt, dk * 128:(dk + 1) * 128],
                        identf[:, :])
                    nc.vector.tensor_copy(
                        out=x2T[:, dk, tt * 128:(tt + 1) * 128],
                        in_=ptr[:, :128])

            gam_sb = consts.tile([128, 8], f32, tag="gam")
            nc.sync.dma_start(out=gam_sb[:, :], in_=gamr[:, :])
            bet_sb = consts.tile([128, 8], f32, tag="bet")
            nc.sync.dma_start(out=bet_sb[:, :], in_=betr[:, :])
            b2_sb = consts.tile([128, NPASS, 8], f32, tag="b2")
            nc.sync.dma_start(out=b2_sb[:, :, :], in_=b2r[:, :, :])

            h2T = acts.tile([128, 8, 512], bf16, tag="bigE")
            for dk in range(8):
                th = tmp.tile([128, 512], f32, tag="t5")
                nc.vector.tensor_tensor(out=th[:, :], in0=x2T[:, dk, :],
                                        in1=rstd2B[:, :], op=OP.mult)
                nc.vector.tensor_scalar(
                    out=h2T[:, dk, :], in0=th[:, :],
                    scalar1=gam_sb[:, dk:dk + 1], scalar2=bet_sb[:, dk:dk + 1],
                    op0=OP.mult, op1=OP.add)

            # ---------------- phase C3: rotation passes
            rAB = acts.tile([128, 2, 8, 512], bf16, tag="bigA")
            cur = h2T
            for p in range(NPASS):
                gsb = gmp.tile([128, 8, D], bf16, tag="g")
                nc.sync.dma_start(
                    out=gsb[:, :, :],
                    in_=gm[p, :, :].rearrange("(ki pp) j -> pp ki j", pp=128))
                for jc in range(8):
                    pr = ps_mm.tile([128, 512], f32, tag="mm")
                    for ki in range(8):
                        nc.tensor.matmul(
                            pr[:, :], gsb[:, ki, jc * 128:(jc + 1) * 128],
                            cur[:, ki, :] if p == 0
                            else cur[:, p & 1 ^ 1, ki, :],
                            start=(ki == 0), stop=(ki == 7))
                    dst_half = p & 1
                    nc.scalar.activation(out=rAB[:, dst_half, jc, :],
                                         in_=pr[:, :], func=AF.Silu,
                                         bias=b2_sb[:, p, jc:jc + 1])
                cur = rAB
            # after 3 passes result is in rAB[:, 0] (p=2 wrote half 0)

            # ---------------- phase C4: y = x2 + r - h2  (T layout out)
            for dk in range(8):
                ty = tmp.tile([128, 512], f32, tag="t5")
                nc.vector.tensor_tensor(out=ty[:, :], in0=rAB[:, 0, dk, :],
                                        in1=h2T[:, dk, :], op=OP.subtract)
                nc.vector.tensor_tensor(out=ty[:, :], in0=ty[:, :],
                                        in1=x2T[:, dk, :], op=OP.add)
                nc.sync.dma_start(out=yt[dk * 128:(dk + 1) * 128, :],
                                  in_=ty[:, :])
    nc.finalize()
    return nc


_NC_CACHE = [None]


def _make_in_maps(x, gamma1, gamma2, scale_beta, qkv_w, o_w, angles, gate,
                  bias, pi, pj):
    import ml_dtypes
    bf = ml_dtypes.bfloat16

    xf = x.reshape(TOK, D).astype(np.float32)
    xt = np.ascontiguousarray(xf.T).astype(bf)

    Wg = qkv_w * gamma1[None, :]
    bW = qkv_w @ scale_beta
    gmats = _giv_mats(angles, pi, pj, gate)
    gm = np.stack(gmats).astype(bf)
    owt = np.ascontiguousarray(o_w.T).astype(bf)
    b2r = np.ascontiguousarray(
        bias.astype(np.float32).reshape(NPASS, 8, 128).transpose(2, 0, 1))
    gamr = np.ascontiguousarray(gamma2.astype(np.float32).reshape(8, 128).T)
    betr = np.ascontiguousarray(
        scale_beta.astype(np.float32).reshape(8, 128).T)

    shared = {"xt": xt, "owt": owt, "gm": gm, "b2r": b2r, "gamr": gamr,
              "betr": betr}
    in_maps = []
    for c in range(NC):
        rq = slice(128 * c, 128 * (c + 1))
        rk = slice(D + 128 * c, D + 128 * (c + 1))
        rv = slice(2 * D + 128 * c, 2 * D + 128 * (c + 1))
        wg_slice = np.concatenate(
            [Wg[rq], Wg[rk] / np.sqrt(HD), Wg[rv]], axis=0)
        bw_slice = np.concatenate(
            [bW[rq], bW[rk] / np.sqrt(HD), bW[rv]], axis=0)
        m = dict(shared)
        m["wgt"] = np.ascontiguousarray(wg_slice.T).astype(bf)
        m["bwr"] = np.ascontiguousarray(
            bw_slice.astype(np.float32).reshape(3, 128).T)
        m["xs"] = np.ascontiguousarray(xf[c * TPC:(c + 1) * TPC])
        in_maps.append(m)
    return in_maps


def _device_run(x, gamma1, gamma2, scale_beta, qkv_w, o_w, angles, gate,
                bias, pi, pj):
    sys.path.insert(0, "/opt/trn_rl_repo")
    from concourse import bass_utils

    if _NC_CACHE[0] is None:
        _NC_CACHE[0] = _build()
    nc = _NC_CACHE[0]
    in_maps = _make_in_maps(x, gamma1, gamma2, scale_beta, qkv_w, o_w,
                            angles, gate, bias, pi, pj)
    res = bass_utils.run_bass_kernel_spmd(nc, in_maps,
                                          core_ids=list(range(NC)))
    yf = np.empty((TOK, D), np.float32)
    for c in range(NC):
        yf[c * TPC:(c + 1) * TPC] = res.results[c]["yt"].T
    return yf.reshape(B, T, D)


def kernel(x, scale_gamma, scale_beta, qkv_w, o_w, norm1_w, norm2_w,
           angles, gate, bias, pi, pj):
    x = np.asarray(x, np.float32)
    args = (np.asarray(scale_gamma, np.float32),
            np.asarray(scale_beta, np.float32),
            np.asarray(qkv_w, np.float32), np.asarray(o_w, np.float32))
    rot = (np.asarray(angles, np.float32), np.asarray(gate, np.float32),
           np.asarray(bias, np.float32), np.asarray(pi), np.asarray(pj))
    g1 = args[0] * np.asarray(norm1_w, np.float32)
    g2 = args[0] * np.asarray(norm2_w, np.float32)
    try:
        return _device_run(x, g1, g2, args[1], args[2], args[3],
                           rot[0], rot[1], rot[2], rot[3], rot[4])
    except Exception as e:  # pragma: no cover - safety net
        print(f"device path failed ({type(e).__name__}: {e}); "
              "using host fallback", file=sys.stderr)
        return _host_fallback(x, args[0], args[1], args[2], args[3],
                              np.asarray(norm1_w, np.float32),
                              np.asarray(norm2_w, np.float32),
                              rot[0], rot[1], rot[2], rot[3], rot[4])
